# revision 30
# baseline (speedup 1.0000x reference)
"""HMM scaled-forward (alpha scaling) kernel for Trainium2, 8 NeuronCores.

Math: alpha_t = normalize((alpha_{t-1} @ A) * b[:, x_t]).
The map v -> normalize((v @ A) * e) is a Hilbert-metric contraction, so the
T=1M scan is split into B=4096 short chains per core (L=31 steps), each
seeded by a host-side fp64 warmup. Per-step normalization is dropped on
device; rows are normalized on the host at the end.

Memory-lean encoding (target_regime=memory):
- Emissions are pre-gathered on the host, prescaled to mean ~SC, and sent as
  fp8-e4m3 (1B/value).  The transition matrix is scaled by 1/SC so the state
  magnitude random-walks around 1.
- Device per step: for each of PAIRS=2 pipelined lines, QP=2 independent
  64-chain... 2x512-chain groups run: two matmuls (PE, shared bf16 weights)
  write one [128, 2, 512] PSUM tile (2 banks), then ONE DVE tensor_mul
  multiplies by the fp8 emissions into bf16 SBUF (contiguous 1024-elem runs).
  The bf16 product tile IS both the next-step state and the DMA'd output
  (state-major layout, contiguous multi-MB transfers, no transposes).
- Host divides out the fp8 emission and applies the exact f32 emission:
  row_t = v_t * (b[:, x_t] / fp8(bs)[:, x_t]), then normalizes.  bf16
  rounding of v=u*e preserves u's relative accuracy, so output error stays
  at bf16 + fp8-steady-state level (~0.7% << 2e-2 tolerance).
"""

import sys

sys.path.insert(0, "/opt/trn_rl_repo")

import numpy as np
import ml_dtypes

# ---- hardcoded geometry (from the problem spec) ----
Y = 64
XV = 50000
T = 1_000_000
NCORES = 8
TCORE = T // NCORES  # 125000

PAIRS = 2               # independent pipelined lines (PE<->DVE overlap)
QP = 3                  # groups per line, fused into one PSUM/DVE op
GRP = PAIRS * QP        # 6 groups total
F = 512                 # chain-pairs per group (matmul moving cols)
B = GRP * 2 * F         # 6144 chains per core
L = 21                  # steps per chain; B*L = 129024 >= TCORE
WINDOWS = [2, 7, 7, 5]  # E-prefetch windows (small first window for a fast head)
BL = B * L              # padded output rows per core
WARM = 32               # host warmup steps (truncated for early chains)
SC = 64.0               # emission prescale (A scaled by 1/SC to compensate)

assert sum(WINDOWS) == L and B * L >= TCORE

F8 = ml_dtypes.float8_e4m3
BF16 = ml_dtypes.bfloat16

LAST_RESULTS = None  # stashed BassKernelResults for test harness introspection

_CACHED_NC = None


def _build_bass():
    import concourse.tile as tile
    from concourse import bacc, mybir
    from contextlib import ExitStack

    f32 = mybir.dt.float32
    bf16 = mybir.dt.bfloat16
    f8 = mybir.dt.float8e4
    nc = bacc.Bacc("TRN2", target_bir_lowering=False)

    # E[p, :, t, q, f]: emissions for line p, group grp=QP*p+q, fused-contiguous
    E = nc.dram_tensor("E", [PAIRS, 128, L, QP, F], f8, kind="ExternalInput")
    # CONST = [AB (128) | V (GRP*F)] packed so the kernel head issues a
    # single DMA wait (LDWEIGHTS tolerates only one sync wait).
    CONST = nc.dram_tensor("CONST", [128, 128 + GRP * F], bf16, kind="ExternalInput")
    OUT = nc.dram_tensor("OUT", [PAIRS, 128, L * QP * F], bf16, kind="ExternalOutput")

    with tile.TileContext(nc) as tc, ExitStack() as ctx:
        singles = ctx.enter_context(tc.tile_pool(name="singles", bufs=1))
        hist_p = ctx.enter_context(tc.tile_pool(name="hist", bufs=4))
        e_p = ctx.enter_context(tc.tile_pool(name="ebuf", bufs=2))
        ps_rec = ctx.enter_context(tc.tile_pool(name="psrec", bufs=2, space="PSUM"))

        const_sb = singles.tile([128, 128 + GRP * F], bf16)
        nc.sync.dma_start(const_sb[:], CONST[:])
        ab_sb = const_sb[:, 0:128]

        # s_prev[p][q]: state AP of group grp=QP*p+q
        s_prev = [
            [
                const_sb[:, 128 + (QP * p + q) * F : 128 + (QP * p + q + 1) * F]
                for q in range(QP)
            ]
            for p in range(PAIRS)
        ]
        w0 = 0
        for kw in WINDOWS:
            e_bufs = []
            for p in range(PAIRS):
                eb = e_p.tile([128, kw, QP, F], f8, tag=f"ebuf{p}")
                (nc.sync if p == 0 else nc.scalar).dma_start(
                    eb[:], E[p, :, w0 : w0 + kw, :, :]
                )
                e_bufs.append(eb)
            for s in range(kw):
                for p in range(PAIRS):
                    ps = ps_rec.tile([128, QP, F], f32, tag="ps")
                    for q in range(QP):
                        nc.tensor.matmul(ps[:, q, :], ab_sb, s_prev[p][q])
                    hv = hist_p.tile([128, QP, F], bf16, tag=f"hv{p}")
                    nc.vector.tensor_mul(
                        out=hv[:],
                        in0=ps[:],
                        in1=e_bufs[p][:, s, :, :],
                    )
                    # per-step contiguous flush from the idle ACT HWDGE queue
                    # (overlaps the recurrence; leaves only ~one step of tail)
                    t = w0 + s
                    (nc.scalar if p == 0 else nc.sync).dma_start(
                        OUT[p, :, QP * F * t : QP * F * (t + 1)], hv[:]
                    )
                    for q in range(QP):
                        s_prev[p][q] = hv[:, q, :]
            w0 += kw
    nc.compile()
    return nc


def _chain_starts():
    """Global t of each chain's first device output row, per core.

    Chain (core 0, c=0) is shifted by one: it is seeded with the exact
    alpha_0 and its rows cover t=1..L (row 0 is computed exactly on the
    host), avoiding the ill-conditioned pi-seed solve."""
    starts = np.empty((NCORES, B), np.int64)
    for k in range(NCORES):
        starts[k] = k * TCORE + np.arange(B) * L
    starts[0, 0] = 1
    return starts


def _prepare_inputs(x, transition, b, pi):
    """Host-side planning: emission pre-gather (fp8), chain seeds, constants."""
    A64 = transition.astype(np.float64)
    bs_f8 = (b.astype(np.float64) * (XV * SC)).astype(F8)  # fp8 prescaled emissions

    pad = ((NCORES - 1) * TCORE + BL + 1) - T
    x_pad = np.concatenate([x, np.repeat(x[-1:], pad)]).astype(np.int64)

    starts = _chain_starts()
    flat_starts = starts.ravel()

    # ---- chain seeds: v_c ~ alpha_{start-1}; device step yields alpha_start
    # Warmup emissions before t=0 are replaced by ones (pure-mixing steps),
    # so chains starting before t=WARM still converge from the prior.
    Vv = np.ones((NCORES * B, Y), np.float64) / Y
    bT64 = np.ascontiguousarray(b.astype(np.float64).T)  # (XV, Y)
    warm_mask = flat_starts > 1  # all chains except (0,0)
    widx = np.empty((int(warm_mask.sum()), WARM), np.int64)
    widx[:] = flat_starts[warm_mask, None] - WARM + np.arange(WARM)[None, :]
    EW = bT64[x_pad[np.maximum(widx, 0)]]  # (M, WARM, Y)
    EW[widx < 0] = 1.0
    Vw = Vv[warm_mask]
    for s in range(WARM):
        Vw = (Vw @ A64) * EW[:, s, :]
        Vw /= Vw.sum(1, keepdims=True)
    Vv[warm_mask] = Vw
    # chain (0,0): seeded with exact alpha_0 (its first device row is t=1)
    a0 = bT64[x_pad[0]] * pi.astype(np.float64)
    Vv[0] = a0 / a0.sum()
    Vv = Vv.astype(BF16).reshape(NCORES, B, Y)

    ABm = np.zeros((128, 128), np.float64)
    ABm[:64, :64] = A64 / SC
    ABm[64:, 64:] = A64 / SC
    ABm = np.ascontiguousarray(ABm.astype(BF16))

    # ---- per-core emission streams:
    # E[p, g*64+j, s, q, f] = bs_f8[j, x[start(c) + s]],  c = ((QP*p+q)*2+g)*F + f
    in_maps = []
    for k in range(NCORES):
        idx = starts[k][:, None] + np.arange(L)[None, :]
        tok = x_pad[idx]  # (B, L) token ids
        Ek = np.empty((PAIRS, 128, L, QP, F), F8)
        for p in range(PAIRS):
            for q in range(QP):
                for g in range(2):
                    c0 = ((QP * p + q) * 2 + g) * F
                    tg = np.ascontiguousarray(tok[c0 : c0 + F].T)  # (L, F)
                    Ek[p, g * 64 : (g + 1) * 64, :, q, :] = np.take(
                        bs_f8, tg.ravel(), axis=1
                    ).reshape(64, L, F)
        Ck = np.empty((128, 128 + GRP * F), BF16)
        Ck[:, 0:128] = ABm
        for grp in range(GRP):
            for g in range(2):
                c0 = (grp * 2 + g) * F
                Ck[g * 64 : (g + 1) * 64, 128 + grp * F : 128 + (grp + 1) * F] = Vv[
                    k, c0 : c0 + F
                ].T
        in_maps.append({"E": Ek, "CONST": Ck})
    return in_maps, bs_f8, x_pad


def _assemble(results, b, pi, x):
    """Device bf16 v-states -> exact-emission-corrected normalized rows."""
    bs_f8 = (b.astype(np.float64) * (XV * SC)).astype(F8)
    dev = np.empty((NCORES, B, L, Y), np.float32)
    for k, r in enumerate(results):
        arr = r["OUT"]  # (PAIRS, 128, L*QP*F) bf16, step-major flat
        blk = arr.astype(np.float32).reshape(PAIRS, 2, Y, L, QP, F)  # (p,g,j,s,q,f)
        # chain c = ((QP*p+q)*2+g)*F+f ; dev[k, c, s, j]
        dev[k] = blk.transpose(0, 4, 1, 5, 3, 2).reshape(B, L, Y)

    full = np.empty((T, Y), np.float32)
    for k in range(NCORES):
        rows = dev[k].reshape(BL, Y)[:TCORE]
        full[k * TCORE : (k + 1) * TCORE] = rows
    # core-0 chain-0 shift: its row s is really t = s+1
    full[1:L] = dev[0, 0, 0 : L - 1, :]

    # divide out the fp8 emission, apply the exact one
    bs_f32 = bs_f8.astype(np.float32)
    ratio = np.where(bs_f32 > 0, b.astype(np.float32) / np.maximum(bs_f32, 1e-30), 0.0)
    full *= ratio.T[x]  # (T, Y) * gather
    # row 0 exact on host
    a0 = b[:, x[0]].astype(np.float64) * pi.astype(np.float64)
    full[0] = (a0 / a0.sum()).astype(np.float32)
    full /= full.sum(axis=1, keepdims=True)
    return full.astype(np.float32)


def kernel(x, transition, b, pi):
    global LAST_RESULTS, _CACHED_NC
    from concourse.bass_utils import run_bass_kernel_spmd

    x = np.asarray(x)
    transition = np.asarray(transition)
    b = np.asarray(b)
    pi = np.asarray(pi)

    in_maps, bs_f8, x_pad = _prepare_inputs(x, transition, b, pi)
    if _CACHED_NC is None:
        _CACHED_NC = _build_bass()
    res = run_bass_kernel_spmd(_CACHED_NC, in_maps, core_ids=list(range(NCORES)))
    LAST_RESULTS = res

    return _assemble(res.results, b, pi, x)


# revision 31
# speedup vs baseline: 1.2161x; 1.2161x over previous
"""HMM scaled-forward (alpha scaling) kernel for Trainium2, 8 NeuronCores.

Math: alpha_t = normalize((alpha_{t-1} @ A) * b[:, x_t]).
The map v -> normalize((v @ A) * e) is a Hilbert-metric contraction, so the
T=1M scan is split into B=4096 short chains per core (L=31 steps), each
seeded by a host-side fp64 warmup. Per-step normalization is dropped on
device; rows are normalized on the host at the end.

Memory-lean encoding (target_regime=memory):
- Emissions are pre-gathered on the host, prescaled to mean ~SC, and sent as
  fp8-e4m3 (1B/value).  The transition matrix is scaled by 1/SC so the state
  magnitude random-walks around 1.
- Device per step: for each of PAIRS=2 pipelined lines, QP=2 independent
  64-chain... 2x512-chain groups run: two matmuls (PE, shared bf16 weights)
  write one [128, 2, 512] PSUM tile (2 banks), then ONE DVE tensor_mul
  multiplies by the fp8 emissions into bf16 SBUF (contiguous 1024-elem runs).
  The bf16 product tile IS both the next-step state and the DMA'd output
  (state-major layout, contiguous multi-MB transfers, no transposes).
- Host divides out the fp8 emission and applies the exact f32 emission:
  row_t = v_t * (b[:, x_t] / fp8(bs)[:, x_t]), then normalizes.  bf16
  rounding of v=u*e preserves u's relative accuracy, so output error stays
  at bf16 + fp8-steady-state level (~0.7% << 2e-2 tolerance).
"""

import sys

sys.path.insert(0, "/opt/trn_rl_repo")

import numpy as np
import ml_dtypes

# ---- hardcoded geometry (from the problem spec) ----
Y = 64
XV = 50000
T = 1_000_000
NCORES = 8
TCORE = T // NCORES  # 125000

PAIRS = 2               # independent pipelined lines (PE<->DVE overlap)
QP = 3                  # groups per line, fused into one PSUM/DVE op
GRP = PAIRS * QP        # 6 groups total
F = 512                 # chain-pairs per group (matmul moving cols)
B = GRP * 2 * F         # 6144 chains per core
L = 21                  # steps per chain; B*L = 129024 >= TCORE
WINDOWS = [2, 7, 7, 5]  # E-prefetch windows (small first window for a fast head)
BL = B * L              # padded output rows per core
WARM = 32               # host warmup steps (truncated for early chains)
SC = 64.0               # emission prescale (A scaled by 1/SC to compensate)

assert sum(WINDOWS) == L and B * L >= TCORE

F8 = ml_dtypes.float8_e4m3
BF16 = ml_dtypes.bfloat16

LAST_RESULTS = None  # stashed BassKernelResults for test harness introspection

_CACHED_NC = None


def _build_bass():
    import concourse.tile as tile
    from concourse import bacc, mybir
    from contextlib import ExitStack

    f32 = mybir.dt.float32
    bf16 = mybir.dt.bfloat16
    f8 = mybir.dt.float8e4
    nc = bacc.Bacc("TRN2", target_bir_lowering=False)

    # E[p, :, t, q, f]: emissions for line p, group grp=QP*p+q, fused-contiguous
    E = nc.dram_tensor("E", [PAIRS, 128, L, QP, F], f8, kind="ExternalInput")
    # CONST = [AB (128) | V (GRP*F)] packed so the kernel head issues a
    # single DMA wait (LDWEIGHTS tolerates only one sync wait).
    CONST = nc.dram_tensor("CONST", [128, 128 + GRP * F], bf16, kind="ExternalInput")
    OUT = nc.dram_tensor("OUT", [PAIRS, 128, L * QP * F], bf16, kind="ExternalOutput")

    with tile.TileContext(nc) as tc, ExitStack() as ctx:
        singles = ctx.enter_context(tc.tile_pool(name="singles", bufs=1))
        hist_p = ctx.enter_context(tc.tile_pool(name="hist", bufs=4))
        e_p = ctx.enter_context(tc.tile_pool(name="ebuf", bufs=2))
        ps_rec = ctx.enter_context(tc.tile_pool(name="psrec", bufs=2, space="PSUM"))

        const_sb = singles.tile([128, 128 + GRP * F], bf16)
        nc.sync.dma_start(const_sb[:], CONST[:])
        ab_sb = const_sb[:, 0:128]

        # s_prev[p][q]: state AP of group grp=QP*p+q
        s_prev = [
            [
                const_sb[:, 128 + (QP * p + q) * F : 128 + (QP * p + q + 1) * F]
                for q in range(QP)
            ]
            for p in range(PAIRS)
        ]
        w0 = 0
        for kw in WINDOWS:
            e_bufs = []
            for p in range(PAIRS):
                eb = e_p.tile([128, kw, QP, F], f8, tag=f"ebuf{p}")
                nc.sync.dma_start(eb[:], E[p, :, w0 : w0 + kw, :, :])
                e_bufs.append(eb)
            for s in range(kw):
                for p in range(PAIRS):
                    ps = ps_rec.tile([128, QP, F], f32, tag="ps")
                    for q in range(QP):
                        nc.tensor.matmul(ps[:, q, :], ab_sb, s_prev[p][q])
                    hv = hist_p.tile([128, QP, F], bf16, tag=f"hv{p}")
                    nc.vector.tensor_mul(
                        out=hv[:],
                        in0=ps[:],
                        in1=e_bufs[p][:, s, :, :],
                    )
                    # per-step contiguous flush from the idle ACT HWDGE queue
                    # (overlaps the recurrence; leaves only ~one step of tail)
                    t = w0 + s
                    nc.scalar.dma_start(OUT[p, :, QP * F * t : QP * F * (t + 1)], hv[:])
                    for q in range(QP):
                        s_prev[p][q] = hv[:, q, :]
            w0 += kw
    nc.compile()
    return nc


def _chain_starts():
    """Global t of each chain's first device output row, per core.

    Chain (core 0, c=0) is shifted by one: it is seeded with the exact
    alpha_0 and its rows cover t=1..L (row 0 is computed exactly on the
    host), avoiding the ill-conditioned pi-seed solve."""
    starts = np.empty((NCORES, B), np.int64)
    for k in range(NCORES):
        starts[k] = k * TCORE + np.arange(B) * L
    starts[0, 0] = 1
    return starts


def _prepare_inputs(x, transition, b, pi):
    """Host-side planning: emission pre-gather (fp8), chain seeds, constants."""
    A64 = transition.astype(np.float64)
    bs_f8 = (b.astype(np.float64) * (XV * SC)).astype(F8)  # fp8 prescaled emissions

    pad = ((NCORES - 1) * TCORE + BL + 1) - T
    x_pad = np.concatenate([x, np.repeat(x[-1:], pad)]).astype(np.int64)

    starts = _chain_starts()
    flat_starts = starts.ravel()

    # ---- chain seeds: v_c ~ alpha_{start-1}; device step yields alpha_start
    # Warmup emissions before t=0 are replaced by ones (pure-mixing steps),
    # so chains starting before t=WARM still converge from the prior.
    Vv = np.ones((NCORES * B, Y), np.float64) / Y
    bT64 = np.ascontiguousarray(b.astype(np.float64).T)  # (XV, Y)
    warm_mask = flat_starts > 1  # all chains except (0,0)
    widx = np.empty((int(warm_mask.sum()), WARM), np.int64)
    widx[:] = flat_starts[warm_mask, None] - WARM + np.arange(WARM)[None, :]
    EW = bT64[x_pad[np.maximum(widx, 0)]]  # (M, WARM, Y)
    EW[widx < 0] = 1.0
    Vw = Vv[warm_mask]
    for s in range(WARM):
        Vw = (Vw @ A64) * EW[:, s, :]
        Vw /= Vw.sum(1, keepdims=True)
    Vv[warm_mask] = Vw
    # chain (0,0): seeded with exact alpha_0 (its first device row is t=1)
    a0 = bT64[x_pad[0]] * pi.astype(np.float64)
    Vv[0] = a0 / a0.sum()
    Vv = Vv.astype(BF16).reshape(NCORES, B, Y)

    ABm = np.zeros((128, 128), np.float64)
    ABm[:64, :64] = A64 / SC
    ABm[64:, 64:] = A64 / SC
    ABm = np.ascontiguousarray(ABm.astype(BF16))

    # ---- per-core emission streams:
    # E[p, g*64+j, s, q, f] = bs_f8[j, x[start(c) + s]],  c = ((QP*p+q)*2+g)*F + f
    in_maps = []
    for k in range(NCORES):
        idx = starts[k][:, None] + np.arange(L)[None, :]
        tok = x_pad[idx]  # (B, L) token ids
        Ek = np.empty((PAIRS, 128, L, QP, F), F8)
        for p in range(PAIRS):
            for q in range(QP):
                for g in range(2):
                    c0 = ((QP * p + q) * 2 + g) * F
                    tg = np.ascontiguousarray(tok[c0 : c0 + F].T)  # (L, F)
                    Ek[p, g * 64 : (g + 1) * 64, :, q, :] = np.take(
                        bs_f8, tg.ravel(), axis=1
                    ).reshape(64, L, F)
        Ck = np.empty((128, 128 + GRP * F), BF16)
        Ck[:, 0:128] = ABm
        for grp in range(GRP):
            for g in range(2):
                c0 = (grp * 2 + g) * F
                Ck[g * 64 : (g + 1) * 64, 128 + grp * F : 128 + (grp + 1) * F] = Vv[
                    k, c0 : c0 + F
                ].T
        in_maps.append({"E": Ek, "CONST": Ck})
    return in_maps, bs_f8, x_pad


def _assemble(results, b, pi, x):
    """Device bf16 v-states -> exact-emission-corrected normalized rows."""
    bs_f8 = (b.astype(np.float64) * (XV * SC)).astype(F8)
    dev = np.empty((NCORES, B, L, Y), np.float32)
    for k, r in enumerate(results):
        arr = r["OUT"]  # (PAIRS, 128, L*QP*F) bf16, step-major flat
        blk = arr.astype(np.float32).reshape(PAIRS, 2, Y, L, QP, F)  # (p,g,j,s,q,f)
        # chain c = ((QP*p+q)*2+g)*F+f ; dev[k, c, s, j]
        dev[k] = blk.transpose(0, 4, 1, 5, 3, 2).reshape(B, L, Y)

    full = np.empty((T, Y), np.float32)
    for k in range(NCORES):
        rows = dev[k].reshape(BL, Y)[:TCORE]
        full[k * TCORE : (k + 1) * TCORE] = rows
    # core-0 chain-0 shift: its row s is really t = s+1
    full[1:L] = dev[0, 0, 0 : L - 1, :]

    # divide out the fp8 emission, apply the exact one
    bs_f32 = bs_f8.astype(np.float32)
    ratio = np.where(bs_f32 > 0, b.astype(np.float32) / np.maximum(bs_f32, 1e-30), 0.0)
    full *= ratio.T[x]  # (T, Y) * gather
    # row 0 exact on host
    a0 = b[:, x[0]].astype(np.float64) * pi.astype(np.float64)
    full[0] = (a0 / a0.sum()).astype(np.float32)
    full /= full.sum(axis=1, keepdims=True)
    return full.astype(np.float32)


def kernel(x, transition, b, pi):
    global LAST_RESULTS, _CACHED_NC
    from concourse.bass_utils import run_bass_kernel_spmd

    x = np.asarray(x)
    transition = np.asarray(transition)
    b = np.asarray(b)
    pi = np.asarray(pi)

    in_maps, bs_f8, x_pad = _prepare_inputs(x, transition, b, pi)
    if _CACHED_NC is None:
        _CACHED_NC = _build_bass()
    res = run_bass_kernel_spmd(_CACHED_NC, in_maps, core_ids=list(range(NCORES)))
    LAST_RESULTS = res

    return _assemble(res.results, b, pi, x)
